# revision 63
# baseline (speedup 1.0000x reference)
"""Distributed attention kernel for Trainium2 (8 NeuronCores).

Computes, matching the reference:
    Q = x @ Wq.T + bq; K = x @ Wk.T + bk; V = x @ Wv.T + bv        [B,S,A]
    dots = Q @ K.T (per batch)                                      [B,S,S]
    attn = softmax(dots, axis=0)            # softmax over the BATCH dim
    out  = (attn @ V) @ Wp.T + bp                                   [B,S,F]

v5: the output projection is FUSED into the V projection on the host
(associativity: (attn@V)@Wp^T + bp == attn@(x@(Wp Wv)^T + Wp bv) + bp), so
the gathered "V" is already projected (VP, same [B,S,F] size) and phase C
collapses to a single matmul chain per output block — the post-gather tail
drops from ~110us of PE work to ~66us (no transposes, no attT staging, no
separate output projection).

Sharding: sequence (rows of Q) is split across the 8 cores (256 rows each,
all 4 batches per core, since the batch-softmax couples batches). Each core
computes K^T and V for its 256 rows and all-gathers them (K^T fp16, V bf16).

v4 (final):
- All flow-critical DMAs on the two HWDGE rings (sync + scalar); the gpsimd
  (SWDGE, ~2us/descriptor) stream carries ONLY the collective triggers, so
  gathers are triggered the moment their inputs complete.
- A tiny warmup AllGather issued first absorbs cross-rank dispatch skew and
  the ncfw cold-start so the K gather doesn't pay them.
- Exactly ONE K gather and ONE V gather (3 ops total on the serialized
  collective lane): per-gather lane cost is mostly fixed stages, so fewer,
  larger gathers finish the lane ~30us earlier per avoided op, and the lane
  end gates the whole attn@V + output-projection tail.
- Phase B still consumes the gathered K in two passes (dots b0/b1, then
  b2/b3 + batch softmax) with per-(core,half) readback tiles, so dots
  overlap the V gather.
- sync ring = loads/readbacks, scalar ring = bounce+output writes (each write
  directly follows its producing activation on the same engine); bounce
  writes stay contiguous since they gate the slowest rank's gather trigger,
  readbacks eat the strides since phases B/C have slack to absorb them.
"""

import numpy as np
import ml_dtypes

import concourse.bass as bass
import concourse.tile as tile
from concourse import bacc, mybir
from concourse.bass_utils import run_bass_kernel_spmd

AF = mybir.ActivationFunctionType
F32 = mybir.dt.float32
F16 = mybir.dt.float16
BF16 = mybir.dt.bfloat16

B, S, F, A = 4, 2048, 1024, 1024
NCORES = 8
SQ = S // NCORES          # 256 q rows per core
BQ = B * SQ               # 1024: batch folded into the free dim
NFT = F // 128            # 8 f-tiles
NAT = A // 128            # 8 a-tiles
NKT = S // 128            # 16 global k-tiles
RG = [list(range(NCORES))]


def build():
    nc = bacc.Bacc("TRN2", target_bir_lowering=False, debug=False)

    xt_ext = nc.declare_dram_parameter("xt", [F, BQ], F16, isOutput=False)
    wqt_ext = nc.declare_dram_parameter("wqt", [F, A], F16, isOutput=False)
    wvt_ext = nc.declare_dram_parameter("wvt", [F, A], F16, isOutput=False)
    bq_ext = nc.declare_dram_parameter("bq", [128, NAT], F32, isOutput=False)
    bk_ext = nc.declare_dram_parameter("bk", [128, NAT], F32, isOutput=False)
    bv_ext = nc.declare_dram_parameter("bv", [1, A], F16, isOutput=False)
    bp_ext = nc.declare_dram_parameter("bp", [1, F], BF16, isOutput=False)
    ones_h_ext = nc.declare_dram_parameter("ones_h", [1, 128], F16, isOutput=False)
    ones_b_ext = nc.declare_dram_parameter("ones_b", [1, 128], BF16, isOutput=False)
    sv_ext = nc.declare_dram_parameter("sv", [128, BQ], F32, isOutput=False)
    kv_ext = nc.declare_dram_parameter("kv", [128, B * NKT], F32, isOutput=False)
    out_ext = nc.declare_dram_parameter("out", [B, SQ, F], F16, isOutput=True)

    with tile.TileContext(nc) as tc:
        with (
            tc.tile_pool(name="dram", bufs=1, space="DRAM") as dram,
            tc.tile_pool(name="live", bufs=1) as live,
            tc.tile_pool(name="main", bufs=1) as main,
        ):
            # ---------------- bounce + gather DRAM buffers ----------------
            warm_in = dram.tile([1, 4], F32, name="warm_in")
            warm_out = dram.tile([NCORES, 1, 4], F32, addr_space="Shared",
                                 name="warm_out")
            # bounce buffers hold the exact SBUF layout the consumers want, so
            # the post-gather readbacks are one contiguous run per partition;
            # the (latency-insensitive) phase-A writes eat the stride instead
            kb = dram.tile([2, 128, NAT * 512], F16, name="kb")
            kg = dram.tile([NCORES, 2, 128, NAT * 512], F16, addr_space="Shared",
                           name="kg")
            # p-major V bounce: readbacks are one contiguous [128,2048] run
            # per (core, f-half); the phase-A writes eat the stride (VAG0 has
            # slack before part1's data gate, the readback does not)
            vbs = [dram.tile([2, 128, 4 * 512], BF16, name=f"vb{i}")
                   for i in range(2)]
            vgs = [dram.tile([NCORES, 2, 128, 4 * 512], BF16,
                             addr_space="Shared", name=f"vg{i}")
                   for i in range(2)]

            # ---------------- whole-kernel SBUF residents ----------------
            bq_sb = live.tile([128, NAT], F32)
            bk_sb = live.tile([128, NAT], F32)
            bv_sb = live.tile([1, A], F16)
            bp_sb = live.tile([1, F], BF16)
            ones_h = live.tile([1, 128], F16)
            ones_b = live.tile([1, 128], BF16)
            sv_sb = live.tile([128, BQ], F32)
            kv_sb = live.tile([128, B * NKT], F32)

            # long-lived across phases
            qt_sb = main.tile([128, NAT * BQ], F16, name="qt")
            W_ = [main.tile([128, BQ], BF16, tag=f"W{kt}", name=f"W{kt}")
                  for kt in range(NKT)]

            # warmup barrier: first thing in the gpsimd stream, absorbs rank
            # dispatch skew off the critical K gather
            nc.gpsimd.collective_compute(
                "AllGather", mybir.AluOpType.bypass, replica_groups=RG,
                ins=[warm_in[:].opt()], outs=[warm_out[:].opt()],
            )

            # ============ phase A: projections (K, V, Q) ============
            psA = tc.tile_pool(name="psA", bufs=1, space="PSUM")
            ps = psA.__enter__()
            with (
                tc.tile_pool(name="phA", bufs=1) as phA,
                tc.tile_pool(name="kout", bufs=4) as kout,
                tc.tile_pool(name="vout", bufs=4) as vout,
            ):
                # loads: sync ring = consts + x->kb bounce + wq; scalar = xt, wv
                xt_sb = phA.tile([128, NFT * BQ], F16, tag="xt", name="xt")
                for ft in range(NFT):
                    nc.scalar.dma_start(
                        xt_sb[:, ft * BQ : (ft + 1) * BQ],
                        xt_ext[ft * 128 : (ft + 1) * 128, :],
                    )
                nc.sync.dma_start(bk_sb[:], bk_ext[:])
                nc.sync.dma_start(bq_sb[:], bq_ext[:])
                nc.sync.dma_start(bv_sb[:], bv_ext[:])
                nc.sync.dma_start(bp_sb[:], bp_ext[:])
                nc.sync.dma_start(ones_h[:], ones_h_ext[:])
                nc.sync.dma_start(ones_b[:], ones_b_ext[:])
                nc.sync.dma_start(sv_sb[:], sv_ext[:])
                nc.sync.dma_start(kv_sb[:], kv_ext[:])

                wv_sb = phA.tile([128, NFT * A], F16, tag="wv", name="wv")
                for ft in range(NFT):
                    nc.scalar.dma_start(
                        wv_sb[:, ft * A : (ft + 1) * A],
                        wvt_ext[ft * 128 : (ft + 1) * 128, :],
                    )
                wq_sb = phA.tile([128, NFT * A], F16, tag="wq", name="wq")
                for ft in range(NFT):
                    nc.sync.dma_start(
                        wq_sb[:, ft * A : (ft + 1) * A],
                        wqt_ext[ft * 128 : (ft + 1) * 128, :],
                    )

                # ---- QK fused: there is no K projection. dots will be
                # computed as (x Wq^T Wk) @ x^T + host-precomputed bias
                # vectors, so the gather ships raw x^T — available the moment
                # xt lands, which starts the K-gather ~40us earlier than any
                # projected tensor could.
                for bh in range(2):
                    for ft in range(NFT):
                        nc.sync.dma_start(
                            kb[bh, :, ft * 512 : ft * 512 + 512],
                            xt_sb[:, ft * BQ + bh * 512 : ft * BQ + bh * 512 + 512],
                        )
                nc.gpsimd.collective_compute(
                    "AllGather", mybir.AluOpType.bypass, replica_groups=RG,
                    ins=[kb[:].opt()], outs=[kg[:].opt()],
                )

                # ---- V projection: psum [128s, 512a] per (b, st, ac).
                # Activations land in p-major SBUF staging so every DRAM
                # bounce write is one contiguous 0.5MiB copy (the strided
                # writes were ~4us each and gated the V gathers' input-wait
                # on the slowest rank).
                for bp in range(2):
                    vstg = [vout.tile([128, 4 * 512], BF16, tag=f"vs{ac}",
                                      name=f"vs{bp}_{ac}") for ac in range(2)]
                    for bl in range(2):
                        b = bp * 2 + bl
                        for st in range(2):
                            for ac in range(2):
                                p = ps.tile([128, 512], F32, tag="pj", bufs=6)
                                for ft in range(NFT):
                                    nc.tensor.matmul(
                                        p[:],
                                        xt_sb[:, ft * BQ + b * SQ + st * 128 : ft * BQ + b * SQ + st * 128 + 128],
                                        wv_sb[:, ft * A + ac * 512 : ft * A + ac * 512 + 512],
                                        start=(ft == 0),
                                        stop=False,
                                    )
                                nc.tensor.matmul(
                                    p[:], ones_h[:], bv_sb[:, ac * 512 : ac * 512 + 512],
                                    start=False, stop=True,
                                )
                                nc.scalar.activation(
                                    vstg[ac][:, (bl * 2 + st) * 512 : (bl * 2 + st) * 512 + 512],
                                    p[:], AF.Copy,
                                )
                    for ac in range(2):
                        nc.scalar.dma_start(vbs[bp][ac], vstg[ac][:])
                    nc.gpsimd.collective_compute(
                        "AllGather", mybir.AluOpType.bypass, replica_groups=RG,
                        ins=[vbs[bp][:].opt()], outs=[vgs[bp][:].opt()],
                    )

                # ---- Q^T projection: psum [128a, 512(b,q)] per (bh, at)
                for bh in range(2):
                    for at in range(NAT):
                        p = ps.tile([128, 512], F32, tag="pj", bufs=6)
                        for ft in range(NFT):
                            nc.tensor.matmul(
                                p[:],
                                wq_sb[:, (ft * NAT + at) * 128 : (ft * NAT + at) * 128 + 128],
                                xt_sb[:, ft * BQ + bh * 512 : ft * BQ + bh * 512 + 512],
                                start=(ft == 0),
                                stop=(ft == NFT - 1),
                            )
                        nc.scalar.activation(
                            qt_sb[:, at * BQ + bh * 512 : at * BQ + bh * 512 + 512],
                            p[:], AF.Identity, bias=bq_sb[:, at : at + 1],
                        )

            psA.__exit__(None, None, None)
            # ============ phase B: dots (fp16) + softmax over batch ============
            # two passes: (1) dots for b0,b1 against the first gathered K half,
            # (2) dots for b2,b3 + the batch softmax once both halves are in
            psB = tc.tile_pool(name="psB", bufs=1, space="PSUM")
            ps = psB.__enter__()
            with (
                tc.tile_pool(name="phB", bufs=1) as phB,
                tc.tile_pool(name="ktp", bufs=8) as ktp,
                tc.tile_pool(name="scr", bufs=3) as scr,
            ):
                E_ = {}
                for bh in range(2):
                    for c in range(NCORES):
                        kc = ktp.tile([128, NAT * 512], F16, tag="kc",
                                      name=f"kc{bh}_{c}")
                        nc.sync.dma_start(kc[:], kg[c, bh])
                        for ktl in range(2):
                            kt = c * 2 + ktl
                            if bh == 0:
                                E_[kt] = phB.tile([128, BQ], BF16,
                                                  tag=f"E{kt}", name=f"E{kt}")
                            E = E_[kt]
                            for bl in range(2):
                                b = bh * 2 + bl
                                p = ps.tile([128, SQ], F32, tag="pd", bufs=8)
                                for at in range(NAT):
                                    nc.tensor.matmul(
                                        p[:],
                                        kc[:, at * 512 + bl * SQ + ktl * 128 : at * 512 + bl * SQ + ktl * 128 + 128],
                                        qt_sb[:, at * BQ + b * SQ : at * BQ + b * SQ + SQ],
                                        start=(at == 0),
                                        stop=(at == NAT - 1),
                                    )
                                # kvec + bq.bk - 30 ride the per-k exp bias;
                                # the per-q term is applied POST-exp as the
                                # multiplicative column scale sv = e^{qv}
                                nc.scalar.activation(
                                    E[:, b * SQ : b * SQ + SQ], p[:], AF.Exp,
                                    bias=kv_sb[:, b * NKT + kt : b * NKT + kt + 1],
                                )
                            if bh == 1:
                                # softmax over batch with the e^{qv} column
                                # scale: attn[b] = E[b] s[b] / sum_b E s
                                T = [scr.tile([128, SQ], F32, tag=f"T{b2}",
                                              name=f"T{kt}_{b2}")
                                     for b2 in range(B)]
                                for b2 in range(B):
                                    nc.gpsimd.tensor_mul(
                                        T[b2][:], E[:, b2 * SQ : b2 * SQ + SQ],
                                        sv_sb[:, b2 * SQ : b2 * SQ + SQ],
                                    )
                                d01 = scr.tile([128, SQ], F32, tag="d01")
                                nc.vector.tensor_add(d01[:], T[0][:], T[1][:])
                                d23 = scr.tile([128, SQ], F32, tag="d23")
                                nc.vector.tensor_add(d23[:], T[2][:], T[3][:])
                                dd = scr.tile([128, SQ], F32, tag="dd")
                                nc.vector.tensor_add(dd[:], d01[:], d23[:])
                                rr = scr.tile([128, SQ], F32, tag="rr")
                                nc.vector.reciprocal_approx_fast(rr[:], dd[:])
                                for b2 in range(B):
                                    nc.vector.tensor_mul(
                                        W_[kt][:, b2 * SQ : b2 * SQ + SQ],
                                        T[b2][:], rr[:],
                                    )


            psB.__exit__(None, None, None)
            # ============ phase C: out = attn @ VP + bp ============
            # The host fused Wp into the V projection (VP = x@(Wp Wv)^T + Wp bv),
            # so the gathered tensor already carries the output projection:
            # one matmul chain per output block, no transposes, no attT.

            psC = tc.tile_pool(name="psC", bufs=1, space="PSUM")
            ps = psC.__enter__()
            with (
                tc.tile_pool(name="vtp", bufs=8) as vtp,
                tc.tile_pool(name="oout", bufs=4) as oout,
            ):
                for bp in range(2):
                    # rank c's VP for this b-pair, both f-halves:
                    # [128k, (bl, st, 512f)], 1KB runs
                    vh_all = {}
                    for fh in range(2):
                        for c in range(NCORES):
                            vh = vtp.tile([128, 2 * 2 * 512], BF16, tag="vh",
                                          name=f"vh{bp}_{fh}_{c}", bufs=16)
                            nc.sync.dma_start(vh[:], vgs[bp][c, fh])
                            vh_all[(fh, c)] = vh
                    for fh in range(2):
                        for bl in range(2):
                            b = bp * 2 + bl
                            for qb in range(2):
                                p = ps.tile([128, 512], F32, tag="po", bufs=6)
                                for c in range(NCORES):
                                    for ktl in range(2):
                                        kt = c * 2 + ktl
                                        nc.tensor.matmul(
                                            p[:],
                                            W_[kt][:, b * SQ + qb * 128 : b * SQ + qb * 128 + 128],
                                            vh_all[(fh, c)][:, (bl * 2 + ktl) * 512 : (bl * 2 + ktl) * 512 + 512],
                                            start=(kt == 0),
                                            stop=False,
                                        )
                                nc.tensor.matmul(
                                    p[:], ones_b[:], bp_sb[:, fh * 512 : fh * 512 + 512],
                                    start=False, stop=True,
                                )
                                ot = oout.tile([128, 512], F16, tag="ot")
                                nc.scalar.activation(ot[:], p[:], AF.Copy)
                                nc.scalar.dma_start(
                                    out_ext[b, qb * 128 : qb * 128 + 128,
                                            fh * 512 : fh * 512 + 512],
                                    ot[:],
                                )
            psC.__exit__(None, None, None)

    nc.finalize()
    return nc


_NC_CACHE = None


def _get_nc():
    global _NC_CACHE
    if _NC_CACHE is None:
        _NC_CACHE = build()
    return _NC_CACHE


def kernel(x, Wq, bq, Wk, bk, Wv, bv, Wp, bp, _trace=False):
    x = np.asarray(x, dtype=np.float32)
    Wq32 = np.asarray(Wq, np.float32)
    Wk32 = np.asarray(Wk, np.float32)
    bq32 = np.asarray(bq, np.float32)
    bk32 = np.asarray(bk, np.float32)
    # fuse Q@K^T:  dots = x (Wq^T Wk) x^T + x(Wq^T bk) + (bq^T Wk)x^T + bq.bk
    # the device projects Q' = x @ M with M = Wq^T Wk and gathers raw x^T;
    # the rank-1 bias cross-terms are precomputed here
    wqt = np.ascontiguousarray(Wq32.T @ Wk32).astype(np.float16)    # M [F,F]
    qv_full = np.einsum("bsf,f->bs", x, Wq32.T @ bk32)              # [B,S]
    kv_full = (np.einsum("bsf,f->bs", x, Wk32.T @ bq32)
               + float(bq32 @ bk32) - 30.0)                         # [B,S]
    kv_p = np.ascontiguousarray(
        kv_full.reshape(B, NKT, 128).transpose(2, 0, 1).reshape(128, B * NKT)
    ).astype(np.float32)
    # fuse the output projection into the V projection (associativity):
    # (attn @ V) @ Wp^T + bp == attn @ (x @ (Wp Wv)^T + Wp bv) + bp
    wpv = np.asarray(Wp, np.float32) @ np.asarray(Wv, np.float32)   # [F, F_in]
    wvt = np.ascontiguousarray(wpv.T).astype(np.float16)
    bq_p = np.zeros((128, NAT), np.float32)   # Q' carries no bias on-device
    bk_p = np.zeros((128, NAT), np.float32)
    bv_p = (np.asarray(Wp, np.float32) @ np.asarray(bv, np.float32)).reshape(
        1, A).astype(np.float16)
    bp_p = np.asarray(bp, np.float32).reshape(1, F).astype(ml_dtypes.bfloat16)
    ones_h = np.ones((1, 128), np.float16)
    ones_b = np.ones((1, 128), ml_dtypes.bfloat16)

    in_maps = []
    for c in range(NCORES):
        # xt: [F, B*SQ] fp16, row f, col (b, q) for this core's q-shard
        xt_c = np.ascontiguousarray(
            x[:, c * SQ : (c + 1) * SQ, :].transpose(2, 0, 1).reshape(F, BQ)
        ).astype(np.float16)
        in_maps.append({
            "xt": xt_c, "wqt": wqt, "wvt": wvt,
            "bq": bq_p, "bk": bk_p, "bv": bv_p, "bp": bp_p,
            "ones_h": ones_h, "ones_b": ones_b,
            "sv": np.ascontiguousarray(np.broadcast_to(
                np.exp(qv_full[:, c * SQ : (c + 1) * SQ].reshape(1, BQ)),
                (128, BQ))).astype(np.float32),
            "kv": kv_p,
        })

    nc = _get_nc()
    res = run_bass_kernel_spmd(
        nc, in_maps, core_ids=list(range(NCORES)), trace=_trace
    )
    out = np.concatenate(
        [np.asarray(res.results[c]["out"], np.float32) for c in range(NCORES)],
        axis=1,
    )
    if _trace:
        kernel.last_results = res
    return out
